# revision 6
# baseline (speedup 1.0000x reference)
"""LDA head (segment-reduce + Mahalanobis scores) on 8 Trainium2 NeuronCores.

Strategy (single SPMD NEFF on 8 cores, fully class-sharded):
  - Core k owns classes [125k, 125k+125). It scans the full batch and computes
    [S1^T | S2^T | counts] = onehot^T @ [z | z^2 | 1] with 16 accumulating PE
    matmuls, then per-class mean, log-prior, and the pooled-variance partial
    r_local = sum_{c in local} sum_{y=c} (z - mu_c)^2  (a 64-vector).
  - The ONLY cross-core data is r: a 512-byte AllReduce(add). A kernel-entry
    barrier AllGather (bir_kernel_barrier_wait) warms the collective engine so
    the mid-kernel AllReduce does not pay the ~20us CC wakeup latency.
  - While the AllReduce is in flight each core builds beta = log prior, the
    transposed stats [mu^T; beta], mu^2^T, and (z^2)^T from a host-supplied
    z^T -- everything that does not depend on the global precision.
  - Post-collective: prec = 1/max(r_tot/TSUM + eps, eps2), V = [prec*mu^T;
    beta - 0.5*m2], q_b = z^2^T @ prec, then 16 matmuls [z^T; 1]^T @ V with
    -0.5*q_b fused as a per-partition activation bias. Output is the
    (2048, 125) column block of the score matrix owned by this core.

kernel(z, y) takes the full inputs and returns the full (2048, 1000) output.
"""

import sys
import numpy as np

if "/opt/trn_rl_repo" not in sys.path:
    sys.path.insert(0, "/opt/trn_rl_repo")

import concourse.bacc as bacc
import concourse.bass as bass
import concourse.mybir as mybir
from concourse import tile
from concourse.bass_utils import run_bass_kernel_spmd

B, C, D = 2048, 1000, 64
NCORES = 8
CL = C // NCORES            # 125 classes per core
NT = B // 128               # 16 batch tiles
EPS_STATS = 1e-5
EPS_PREC = 1e-6
TSUM = float(np.float32(B) + np.float32(C * EPS_STATS))   # counts.sum()
FP = mybir.dt.float32
AF = mybir.ActivationFunctionType
ALU = mybir.AluOpType


def build_program():
    nc = bacc.Bacc("TRN2", target_bir_lowering=False, debug=False,
                   num_devices=NCORES)

    zrow = nc.dram_tensor("zrow", [128, NT, D], FP, kind="ExternalInput")
    ycols = nc.dram_tensor("ycols", [128, NT], FP, kind="ExternalInput")
    cvals = nc.dram_tensor("cvals", [128, CL], FP, kind="ExternalInput")
    zT_in = nc.dram_tensor("zT_in", [D, B], FP, kind="ExternalInput")
    ident = nc.dram_tensor("ident", [128, 128], FP, kind="ExternalInput")
    out = nc.dram_tensor("out_loc", [B, CL], FP, kind="ExternalOutput")

    with tile.TileContext(nc) as tc:
        with tc.tile_pool(name="sb", bufs=1) as pool, \
             tc.tile_pool(name="ps", bufs=8, space="PSUM") as pp, \
             tc.tile_pool(name="dram", bufs=1, space="DRAM") as dr:

            g_in = dr.tile([128, 1], FP)
            g_out = dr.tile([128, 1], FP)

            # ---- input DMAs -------------------------------------------------
            cv = pool.tile([128, CL], FP)
            nc.sync.dma_start(cv[:], cvals[:, :])
            yc = pool.tile([128, NT], FP)
            nc.sync.dma_start(yc[:], ycols[:, :])
            idn = pool.tile([128, 128], FP)
            nc.sync.dma_start(idn[:], ident[:, :])
            M = pool.tile([128, NT, 130], FP)      # [z | z^2 | 1 | pad]
            nc.sync.dma_start(M[:, :, 0:D], zrow[:, :, :])
            # z^T for the score matmuls (65th row of ones for the gamma term)
            zTq = pool.tile([65, B], FP)
            nc.vector.memset(zTq[64:65, :], 1.0)
            nc.sync.dma_start(zTq[0:64, :], zT_in[:, :])

            # ---- phase A: class-sharded segment stats ----------------------
            nc.scalar.activation(M[:, :, D:2 * D], M[:, :, 0:D], AF.Square)
            nc.vector.memset(M[:, :, 2 * D:2 * D + 1], 1.0)

            # all on DVE: gpsimd shares SBUF ports with DVE and slows
            # these ops ~7x when run concurrently
            oh = pool.tile([128, NT, CL], FP)
            for t in range(NT):
                nc.vector.tensor_scalar(oh[:, t, :], cv[:], yc[:, t:t + 1],
                                        None, ALU.is_equal)

            psS = pp.tile([CL, 129], FP, tag="ps")
            for t in range(NT):
                nc.tensor.matmul(psS[:], lhsT=oh[:, t, :], rhs=M[:, t, 0:129],
                                 start=(t == 0), stop=(t == NT - 1))

            # stats post-processing, all in class-partition layout
            cnt = pool.tile([CL, 1], FP)
            nc.vector.tensor_scalar_add(cnt[:], psS[:, 128:129], EPS_STATS)
            rcp = pool.tile([CL, 1], FP)
            nc.vector.reciprocal(rcp[:], cnt[:])

            TB = pool.tile([CL, 65], FP)           # [mean^T | beta]
            nc.vector.tensor_scalar(TB[:, 0:64], psS[:, 0:64], rcp[:], None,
                                    ALU.mult)

            cnt2 = pool.tile([CL, 1], FP)
            nc.vector.tensor_scalar_add(cnt2[:], psS[:, 128:129],
                                        2.0 * EPS_STATS)
            rcp2 = pool.tile([CL, 1], FP)
            nc.vector.tensor_tensor(rcp2[:], rcp[:], rcp[:], ALU.mult)
            alph = pool.tile([CL, 1], FP)
            nc.vector.tensor_tensor(alph[:], cnt2[:], rcp2[:], ALU.mult)

            s1sq = pool.tile([CL, 64], FP)
            nc.scalar.activation(s1sq[:], psS[:, 0:64], AF.Square)
            t1 = pool.tile([CL, 64], FP)
            nc.vector.tensor_scalar(t1[:], s1sq[:], alph[:], None, ALU.mult)
            ptile = pool.tile([CL, 64], FP)
            nc.vector.tensor_tensor(ptile[:], psS[:, 64:128], t1[:],
                                    ALU.subtract)

            # r_local = ptile^T @ ones  -> dim-major [64, 1]
            onesc = pool.tile([CL, 1], FP)
            nc.vector.memset(onesc[:], 1.0)
            psP = pp.tile([64, 1], FP, tag="ps")
            nc.tensor.matmul(psP[:], lhsT=ptile[:], rhs=onesc[:],
                             start=True, stop=True)
            rsb = pool.tile([128, 1], FP)
            nc.vector.memset(rsb[:], 0.0)
            nc.scalar.copy(rsb[0:64, :], psP[:])
            nc.sync.dma_start(g_in[:], rsb[:])

            # ---- tiny AllReduce of the pooled-variance partial --------------
            # Registering a barrier replica group makes compile() insert a
            # 1-byte AllGather at kernel entry; it warms the CC cores during
            # phase A so this AllReduce doesn't pay the ~20us wakeup. No wait
            # needed: NRT rendezvous orders collectives across ranks.
            nc._bir_kernel_barrier_sem_replica_groups.append(set(range(NCORES)))
            nc.gpsimd.collective_compute(
                "AllReduce", ALU.add,
                replica_groups=[list(range(NCORES))],
                ins=[g_in.opt()], outs=[g_out.opt()],
            )

            # ---- prec-independent work, overlaps the collective -------------
            nc.scalar.activation(TB[:, 64:65], cnt[:], AF.Ln,
                                 scale=1.0 / TSUM)
            psT = pp.tile([65, CL], FP, tag="ps")
            nc.tensor.transpose(psT[:], TB[:, :], idn[0:CL, 0:CL])
            sbT = pool.tile([65, CL], FP)          # [mu^T ; beta^T]
            nc.scalar.copy(sbT[:], psT[:])
            # beta^T shifted to partition 0 so the gamma add below has both
            # DVE inputs at the same base partition (HW constraint)
            brow = pool.tile([1, CL], FP)
            nc.sync.dma_start(brow[:], sbT[64:65, :])
            musqT = pool.tile([64, CL], FP)
            nc.vector.tensor_tensor(musqT[:], sbT[0:64, :], sbT[0:64, :],
                                    ALU.mult)
            zsqT = pool.tile([64, B], FP)
            nc.vector.tensor_tensor(zsqT[:], zTq[0:64, :], zTq[0:64, :],
                                    ALU.mult)

            # ---- phase B: scores for all 2048 rows x local 125 classes ------
            rred = pool.tile([64, 1], FP)
            nc.sync.dma_start(rred[:], g_out[0:64, :])
            pooled = pool.tile([64, 1], FP)
            nc.vector.tensor_scalar(pooled[:], rred[:], 1.0 / TSUM,
                                    EPS_STATS, ALU.mult, ALU.add)
            pmax = pool.tile([64, 1], FP)
            nc.vector.tensor_scalar_max(pmax[:], pooled[:], EPS_PREC)
            prec = pool.tile([64, 1], FP)
            nc.vector.reciprocal(prec[:], pmax[:])

            V = pool.tile([65, CL], FP)            # [prec*mu^T ; gamma]
            nc.vector.tensor_scalar(V[0:64, :], sbT[0:64, :], prec[:], None,
                                    ALU.mult)
            psM = pp.tile([1, CL], FP, tag="ps")
            nc.tensor.matmul(psM[:], lhsT=prec[:], rhs=musqT[:],
                             start=True, stop=True)
            nhm2 = pool.tile([1, CL], FP)
            nc.scalar.activation(nhm2[:], psM[:], AF.Copy, scale=-0.5)
            nc.vector.tensor_tensor(V[64:65, :], nhm2[:], brow[:],
                                    ALU.add)

            # q_b = -0.5 * sum_d prec_d z_bd^2, per 128-row tile
            qsb = pool.tile([128, NT], FP)
            for j in range(NT):
                psQ = pp.tile([128, 1], FP, tag="ps")
                nc.tensor.matmul(psQ[:], lhsT=zsqT[:, j * 128:(j + 1) * 128],
                                 rhs=prec[:], start=True, stop=True)
                nc.scalar.activation(qsb[:, j:j + 1], psQ[:], AF.Copy,
                                     scale=-0.5)

            for j in range(NT):
                psO = pp.tile([128, CL], FP, tag="ps")
                nc.tensor.matmul(psO[:], lhsT=zTq[:, j * 128:(j + 1) * 128],
                                 rhs=V[:], start=True, stop=True)
                outj = pool.tile([128, CL], FP, tag=f"outsb{j % 4}")
                nc.scalar.activation(outj[:], psO[:], AF.Identity,
                                     bias=qsb[:, j:j + 1], scale=1.0)
                nc.sync.dma_start(out[j * 128:(j + 1) * 128, :], outj[:])

    nc.compile()
    return nc


_NC_CACHE = None


def _get_program():
    global _NC_CACHE
    if _NC_CACHE is None:
        _NC_CACHE = build_program()
    return _NC_CACHE


def make_in_maps(z, y):
    z = np.ascontiguousarray(np.asarray(z, dtype=np.float32))
    yf = np.asarray(y).astype(np.float32)          # labels < 1000, exact
    zrow_np = z.reshape(128, NT, D)                # row p*16+t -> [p, t, :]
    ycols_np = np.ascontiguousarray(yf.reshape(128, NT))
    zT_np = np.ascontiguousarray(z.T)
    ident_np = np.eye(128, dtype=np.float32)
    in_maps = []
    for k in range(NCORES):
        cvals_np = np.broadcast_to(
            np.arange(k * CL, (k + 1) * CL, dtype=np.float32), (128, CL))
        in_maps.append({
            "zrow": zrow_np,
            "ycols": ycols_np,
            "cvals": np.ascontiguousarray(cvals_np),
            "zT_in": zT_np,
            "ident": ident_np,
        })
    return in_maps


def run(z, y, trace=False, **kwargs):
    nc = _get_program()
    res = run_bass_kernel_spmd(nc, make_in_maps(z, y), list(range(NCORES)),
                               trace=trace, **kwargs)
    full = np.concatenate([res.results[k]["out_loc"] for k in range(NCORES)],
                          axis=1)
    return full, res


def kernel(z, y):
    full, _ = run(z, y, trace=False)
    return full


if __name__ == "__main__":
    rng = np.random.default_rng(0)
    z = rng.standard_normal((B, D), dtype=np.float32)
    y = rng.integers(0, C, size=(B,)).astype(np.int64)
    out = kernel(z, y)
    print("out", out.shape, out.dtype, out[0, :4])


# revision 11
# speedup vs baseline: 1.1094x; 1.1094x over previous
"""LDA head (segment-reduce + Mahalanobis scores) on 8 Trainium2 NeuronCores.

Strategy (single SPMD NEFF on 8 cores, fully class-sharded):
  - Core k owns classes [125k, 125k+125). It scans the full batch and computes
    [S1^T | S2^T | counts] = onehot^T @ [z | z^2 | 1] with 16 accumulating PE
    matmuls, then per-class mean, log-prior, and the pooled-variance partial
    r_local = sum_{c in local} sum_{y=c} (z - mu_c)^2  (a 64-vector).
  - The ONLY cross-core data is r: a 512-byte AllReduce(add). A kernel-entry
    barrier AllGather (bir_kernel_barrier_wait) warms the collective engine so
    the mid-kernel AllReduce does not pay the ~20us CC wakeup latency.
  - While the AllReduce is in flight each core builds beta = log prior, the
    transposed stats [mu^T; beta], mu^2^T, and (z^2)^T from a host-supplied
    z^T -- everything that does not depend on the global precision.
  - Post-collective: prec = 1/max(r_tot/TSUM + eps, eps2), V = [prec*mu^T;
    beta - 0.5*m2], q_b = z^2^T @ prec, then 16 matmuls [z^T; 1]^T @ V with
    -0.5*q_b fused as a per-partition activation bias. Output is the
    (2048, 125) column block of the score matrix owned by this core.

kernel(z, y) takes the full inputs and returns the full (2048, 1000) output.
"""

import sys
import numpy as np

if "/opt/trn_rl_repo" not in sys.path:
    sys.path.insert(0, "/opt/trn_rl_repo")

import concourse.bacc as bacc
import concourse.bass as bass
import concourse.mybir as mybir
from concourse import tile
from concourse.bass_utils import run_bass_kernel_spmd

B, C, D = 2048, 1000, 64
NCORES = 8
CL = C // NCORES            # 125 classes per core
NT = B // 128               # 16 batch tiles
EPS_STATS = 1e-5
EPS_PREC = 1e-6
TSUM = float(np.float32(B) + np.float32(C * EPS_STATS))   # counts.sum()
FP = mybir.dt.float32
AF = mybir.ActivationFunctionType
ALU = mybir.AluOpType


def build_program():
    nc = bacc.Bacc("TRN2", target_bir_lowering=False, debug=False,
                   num_devices=NCORES)

    zrow = nc.dram_tensor("zrow", [128, NT, D], FP, kind="ExternalInput")
    ycols = nc.dram_tensor("ycols", [128, NT], FP, kind="ExternalInput")
    cvals = nc.dram_tensor("cvals", [128, CL], FP, kind="ExternalInput")
    zT_in = nc.dram_tensor("zT_in", [D, B], FP, kind="ExternalInput")
    ident = nc.dram_tensor("ident", [128, 128], FP, kind="ExternalInput")
    out = nc.dram_tensor("out_loc", [B, CL], FP, kind="ExternalOutput")

    with tile.TileContext(nc) as tc:
        with tc.tile_pool(name="sb", bufs=1) as pool, \
             tc.tile_pool(name="ps", bufs=8, space="PSUM") as pp, \
             tc.tile_pool(name="dram", bufs=1, space="DRAM") as dr:

            g_in = dr.tile([128, 1], FP)
            g_out = dr.tile([NCORES, 128, 1], FP, addr_space="Shared")

            # ---- input DMAs -------------------------------------------------
            cv = pool.tile([128, CL], FP)
            nc.sync.dma_start(cv[:], cvals[:, :])
            yc = pool.tile([128, NT], FP)
            nc.sync.dma_start(yc[:], ycols[:, :])
            idn = pool.tile([128, 128], FP)
            nc.sync.dma_start(idn[:], ident[:, :])
            M = pool.tile([128, NT, 130], FP)      # [z | z^2 | 1 | pad]
            nc.sync.dma_start(M[:, :, 0:D], zrow[:, :, :])
            # z^T for the score matmuls (65th row of ones for the gamma term)
            zTq = pool.tile([65, B], FP)
            nc.vector.memset(zTq[64:65, :], 1.0)
            nc.sync.dma_start(zTq[0:64, :], zT_in[:, :])

            # ---- phase A: class-sharded segment stats ----------------------
            nc.scalar.activation(M[:, :, D:2 * D], M[:, :, 0:D], AF.Square)
            nc.vector.memset(M[:, :, 2 * D:2 * D + 1], 1.0)

            # all on DVE: gpsimd shares SBUF ports with DVE and slows
            # these ops ~7x when run concurrently
            oh = pool.tile([128, NT, CL], FP)
            for t in range(NT):
                nc.vector.tensor_scalar(oh[:, t, :], cv[:], yc[:, t:t + 1],
                                        None, ALU.is_equal)

            psS = pp.tile([CL, 129], FP, tag="ps")
            for t in range(NT):
                nc.tensor.matmul(psS[:], lhsT=oh[:, t, :], rhs=M[:, t, 0:129],
                                 start=(t == 0), stop=(t == NT - 1))

            # stats post-processing, all in class-partition layout
            cnt = pool.tile([CL, 1], FP)
            nc.vector.tensor_scalar_add(cnt[:], psS[:, 128:129], EPS_STATS)
            rcp = pool.tile([CL, 1], FP)
            nc.vector.reciprocal(rcp[:], cnt[:])

            TB = pool.tile([CL, 65], FP)           # [mean^T | beta]
            nc.vector.tensor_scalar(TB[:, 0:64], psS[:, 0:64], rcp[:], None,
                                    ALU.mult)

            cnt2 = pool.tile([CL, 1], FP)
            nc.vector.tensor_scalar_add(cnt2[:], psS[:, 128:129],
                                        2.0 * EPS_STATS)
            rcp2 = pool.tile([CL, 1], FP)
            nc.vector.tensor_tensor(rcp2[:], rcp[:], rcp[:], ALU.mult)
            alph = pool.tile([CL, 1], FP)
            nc.vector.tensor_tensor(alph[:], cnt2[:], rcp2[:], ALU.mult)

            s1sq = pool.tile([CL, 64], FP)
            nc.scalar.activation(s1sq[:], psS[:, 0:64], AF.Square)
            t1 = pool.tile([CL, 64], FP)
            nc.vector.tensor_scalar(t1[:], s1sq[:], alph[:], None, ALU.mult)
            ptile = pool.tile([CL, 64], FP)
            nc.vector.tensor_tensor(ptile[:], psS[:, 64:128], t1[:],
                                    ALU.subtract)

            # r_local = ptile^T @ ones  -> dim-major [64, 1]
            onesc = pool.tile([CL, 1], FP)
            nc.vector.memset(onesc[:], 1.0)
            psP = pp.tile([64, 1], FP, tag="ps")
            nc.tensor.matmul(psP[:], lhsT=ptile[:], rhs=onesc[:],
                             start=True, stop=True)
            rsb = pool.tile([128, 1], FP)
            nc.vector.memset(rsb[:], 0.0)
            nc.scalar.copy(rsb[0:64, :], psP[:])
            nc.sync.dma_start(g_in[:], rsb[:])

            # ---- tiny AllGather of the pooled-variance partials -------------
            # (AllGather's mesh is ~2x faster than AllReduce's; the 8-way
            # add is one cheap DVE reduce on the receiver side.)
            nc.gpsimd.collective_compute(
                "AllGather", ALU.bypass,
                replica_groups=[list(range(NCORES))],
                ins=[g_in.opt()], outs=[g_out.opt()],
            )

            # ---- prec-independent work, overlaps the collective -------------
            nc.scalar.activation(TB[:, 64:65], cnt[:], AF.Ln,
                                 scale=1.0 / TSUM)
            psT = pp.tile([65, CL], FP, tag="ps")
            nc.tensor.transpose(psT[:], TB[:, :], idn[0:CL, 0:CL])
            sbT = pool.tile([65, CL], FP)          # [mu^T ; beta^T]
            nc.scalar.copy(sbT[:], psT[:])
            # beta^T shifted to partition 0 so the gamma add below has both
            # DVE inputs at the same base partition (HW constraint)
            brow = pool.tile([1, CL], FP)
            nc.sync.dma_start(brow[:], sbT[64:65, :])
            musqT = pool.tile([64, CL], FP)
            nc.vector.tensor_tensor(musqT[:], sbT[0:64, :], sbT[0:64, :],
                                    ALU.mult)
            onesr = pool.tile([1, 128], FP)
            nc.vector.memset(onesr[:], 1.0)

            # ---- phase B: scores for all 2048 rows x local 125 classes ------
            rpool = pool.tile([64, NCORES], FP)
            nc.sync.dma_start(rpool[:],
                              g_out[:, 0:64, 0].rearrange("k d -> d k"))
            ptot = pool.tile([64, 1], FP)
            nc.vector.reduce_sum(ptot[:], rpool[:], axis=mybir.AxisListType.X)
            pooled = pool.tile([64, 1], FP)
            nc.vector.tensor_scalar(pooled[:], ptot[:], 1.0 / TSUM,
                                    EPS_STATS, ALU.mult, ALU.add)
            pmax = pool.tile([64, 1], FP)
            nc.vector.tensor_scalar_max(pmax[:], pooled[:], EPS_PREC)
            prec = pool.tile([64, 1], FP)
            nc.vector.reciprocal(prec[:], pmax[:])

            V = pool.tile([65, CL], FP)            # [prec*mu^T ; gamma]
            nc.vector.tensor_scalar(V[0:64, :], sbT[0:64, :], prec[:], None,
                                    ALU.mult)
            psM = pp.tile([1, CL], FP, tag="ps")
            nc.tensor.matmul(psM[:], lhsT=prec[:], rhs=musqT[:],
                             start=True, stop=True)
            nhm2 = pool.tile([1, CL], FP)
            nc.scalar.activation(nhm2[:], psM[:], AF.Copy, scale=-0.5)
            nc.vector.tensor_tensor(V[64:65, :], nhm2[:], brow[:],
                                    ALU.add)

            # q_b = -0.5 * sum_d prec_d z_bd^2 on DVE: broadcast -prec/2 to all
            # 128 partitions via a rank-1 PE outer product, then one fused
            # multiply over the row-major z^2 block already in M + one reduce.
            psPR = pp.tile([1, 64], FP, tag="ps")
            nc.tensor.transpose(psPR[:], prec[:], idn[0:64, 0:64])
            prow = pool.tile([1, 64], FP)
            nc.scalar.activation(prow[:], psPR[:], AF.Copy, scale=-0.5)
            psPB = pp.tile([128, 64], FP, tag="ps")
            nc.tensor.matmul(psPB[:], lhsT=onesr[:], rhs=prow[:],
                             start=True, stop=True)
            precbc = pool.tile([128, 64], FP)
            nc.vector.tensor_copy(precbc[:], psPB[:])

            tmpq = pool.tile([128, NT, 64], FP)
            nc.vector.tensor_tensor(
                tmpq[:], M[:, :, 64:128],
                precbc[:, None, :].broadcast_to([128, NT, 64]), ALU.mult)
            qsb = pool.tile([128, NT], FP)
            nc.vector.reduce_sum(qsb[:], tmpq[:], axis=mybir.AxisListType.X)

            for j in range(NT):
                psO = pp.tile([128, CL], FP, tag="ps")
                nc.tensor.matmul(psO[:], lhsT=zTq[:, j * 128:(j + 1) * 128],
                                 rhs=V[:], start=True, stop=True)
                outj = pool.tile([128, CL], FP, tag=f"outsb{j % 4}")
                nc.scalar.activation(outj[:], psO[:], AF.Identity,
                                     bias=qsb[:, j:j + 1], scale=1.0)
                nc.sync.dma_start(out[j * 128:(j + 1) * 128, :], outj[:])

    nc.compile()
    return nc


_NC_CACHE = None


def _get_program():
    global _NC_CACHE
    if _NC_CACHE is None:
        _NC_CACHE = build_program()
    return _NC_CACHE


def make_in_maps(z, y):
    z = np.ascontiguousarray(np.asarray(z, dtype=np.float32))
    yf = np.asarray(y).astype(np.float32)          # labels < 1000, exact
    # [p, t, :] = batch row t*128+p, so qsb[p, t] lines up with score tile t
    zrow_np = np.ascontiguousarray(z.reshape(NT, 128, D).transpose(1, 0, 2))
    ycols_np = np.ascontiguousarray(yf.reshape(NT, 128).T)
    zT_np = np.ascontiguousarray(z.T)
    ident_np = np.eye(128, dtype=np.float32)
    in_maps = []
    for k in range(NCORES):
        cvals_np = np.broadcast_to(
            np.arange(k * CL, (k + 1) * CL, dtype=np.float32), (128, CL))
        in_maps.append({
            "zrow": zrow_np,
            "ycols": ycols_np,
            "cvals": np.ascontiguousarray(cvals_np),
            "zT_in": zT_np,
            "ident": ident_np,
        })
    return in_maps


def run(z, y, trace=False, **kwargs):
    nc = _get_program()
    res = run_bass_kernel_spmd(nc, make_in_maps(z, y), list(range(NCORES)),
                               trace=trace, **kwargs)
    full = np.concatenate([res.results[k]["out_loc"] for k in range(NCORES)],
                          axis=1)
    return full, res


def kernel(z, y):
    full, _ = run(z, y, trace=False)
    return full


if __name__ == "__main__":
    rng = np.random.default_rng(0)
    z = rng.standard_normal((B, D), dtype=np.float32)
    y = rng.integers(0, C, size=(B,)).astype(np.int64)
    out = kernel(z, y)
    print("out", out.shape, out.dtype, out[0, :4])
